# revision 6
# baseline (speedup 1.0000x reference)
"""Trainium2 Bass kernel for a YOLO-style detection loss.

Strategy (data-parallel over batch, per sharding hint):
  - Shard preds on batch dim: 4 images per core across 8 cores; partition
    targets by image index. The three layer shards are concatenated into
    ONE flat [R,11] DRAM tensor per core.
  - Each core gathers ALL of its (target,layer) rows with a SINGLE
    indirect DMA (one descriptor per row; the 994ns SWDGE fixed cost is
    paid once instead of once per 128 rows), then computes the three
    loss partial sums:
      box: DVE (G - T) then abs-reduce (pads have T=0),
      obj/cls-softplus: ACT Softplus with per-partition accumulator
        (one act-table load total; Exp+Ln would need two),
      cls dot with one-hot: DVE mul + reduce (pads have one-hot=0).
    Cross-partition reduction via PE matmul-by-ones (PE is idle).
  - Padding slots gather row 0 of the core's pred shard; the host knows
    row 0 and subtracts the deterministic pad contribution, so no
    per-slot weight tensors are needed on device. Gains and 1/n are
    applied on host when combining the 8 per-core partial quadruples.
"""

import numpy as np

P = 128
NCLS = 6
NO = NCLS + 5
BS = 32
NA = 3
NCORES = 8
BPC = BS // NCORES  # images per core
LAYERS = ((160, 160), (80, 80), (40, 40))  # (ny, nx)
ROWS_PER_LAYER = tuple(BPC * NA * ny * nx for ny, nx in LAYERS)
TOT_ROWS = sum(ROWS_PER_LAYER)  # 403200
ROW_F1, ROW_F2 = 630, 640  # 630*640 == 403200
LAYER_BASE = (0, ROWS_PER_LAYER[0], ROWS_PER_LAYER[0] + ROWS_PER_LAYER[1])
BOX_GAIN, CLS_GAIN, DFL_GAIN = 7.5, 0.5, 1.5

_BUILD_CACHE: dict = {}


def _emit_body(nc, pool, psp, predc, idx_t, aux_t, ones, out_ap, S):
    """Emit one loss-computation body (gather + compute + out DMA).

    Writes [box_raw, obj_raw, clsSP_raw, clsW2_raw] to out_ap ([4,1])."""
    from concourse import bass, mybir

    f32 = mybir.dt.float32
    add = mybir.AluOpType.add
    A = mybir.ActivationFunctionType

    G = pool.tile([P, S * NO], f32, tag="G")
    nc.gpsimd.indirect_dma_start(
        out=G[:, :],
        out_offset=None,
        in_=predc,
        in_offset=bass.IndirectOffsetOnAxis(ap=idx_t[:, 0:S], axis=1),
    )
    G3 = G[:].rearrange("p (s f) -> p s f", f=NO)
    T3 = aux_t[:, 0 : 4 * S].rearrange("p (s f) -> p s f", f=4)
    W23 = aux_t[:, 4 * S : 10 * S].rearrange("p (s f) -> p s f", f=6)

    part = pool.tile([P, 4], f32, tag="part")

    # box: sum |G[:, :, 0:4] - T| (pads: G-T = row0-0, host-corrected)
    D = pool.tile([P, S * 4], f32, tag="D")
    D3 = D[:].rearrange("p (s f) -> p s f", f=4)
    nc.vector.tensor_sub(out=D3, in0=G3[:, :, 0:4], in1=T3)
    nc.vector.tensor_reduce(
        out=part[:, 0:1],
        in_=D3,
        axis=mybir.AxisListType.XY,
        op=add,
        apply_absolute_value=True,
    )

    # obj: sum softplus(-G[:, :, 4]) = sum ln(1 + exp(-x))
    # (Exp and Ln share one act-table set, preloaded once in the prologue)
    scrA = pool.tile([P, S], f32, tag="scrA")
    scrA3 = scrA[:].rearrange("p (s o) -> p s o", o=1)
    nc.scalar.activation(scrA3, G3[:, :, 4:5], A.Exp, scale=-1.0)
    scrB = pool.tile([P, S], f32, tag="scrB")
    scrB3 = scrB[:].rearrange("p (s o) -> p s o", o=1)
    nc.scalar.activation(scrB3, scrA3, A.Ln, bias=1.0)
    nc.vector.tensor_reduce(
        out=part[:, 1:2], in_=scrB[:], axis=mybir.AxisListType.X, op=add
    )

    # cls softplus: sum ln(1 + exp(G[:, :, 5:11]))
    scr6 = pool.tile([P, S * 6], f32, tag="scr6")
    scr63 = scr6[:].rearrange("p (s f) -> p s f", f=6)
    nc.scalar.activation(scr63, G3[:, :, 5:11], A.Exp)
    scr6b = pool.tile([P, S * 6], f32, tag="scr6b")
    scr6b3 = scr6b[:].rearrange("p (s f) -> p s f", f=6)
    nc.scalar.activation(scr6b3, scr63, A.Ln, bias=1.0)
    nc.vector.tensor_reduce(
        out=part[:, 2:3], in_=scr6b[:], axis=mybir.AxisListType.X, op=add
    )

    # cls one-hot dot: sum G[:, :, 5:11] * onehot (pads: onehot=0)
    M = pool.tile([P, S * 6], f32, tag="M")
    M3 = M[:].rearrange("p (s f) -> p s f", f=6)
    nc.vector.tensor_mul(out=M3, in0=G3[:, :, 5:11], in1=W23)
    nc.vector.tensor_reduce(
        out=part[:, 3:4], in_=M3, axis=mybir.AxisListType.XY, op=add
    )

    # cross-partition sum on the (idle) PE: out = part.T @ ones -> [4,1]
    ps = psp.tile([P, 1], f32, tag="ps")
    nc.tensor.matmul(
        out=ps[:4, :1], lhsT=part[:, 0:4], rhs=ones[:, 0:1], start=True, stop=True
    )
    outsb = pool.tile([P, 1], f32, tag="outsb")
    nc.vector.tensor_copy(out=outsb[:4, :1], in_=ps[:4, :1])
    nc.sync.dma_start(out=out_ap, in_=outsb[0:4, 0:1])


def _build(S: int, repeat: int = 1):
    """Build + compile the per-core Bass program for S slots per partition
    (capacity S*128 (target,layer) pairs per core). repeat>1 unrolls the
    body for benchmarking only; the graded path uses repeat=1."""
    from concourse import bacc, mybir, tile

    f32 = mybir.dt.float32

    nc = bacc.Bacc(
        "TRN2",
        target_bir_lowering=False,
        debug=False,
        enable_asserts=False,
        dynamic_dma_scratch_size=131072,
    )

    predc = nc.dram_tensor(
        "predc", [ROW_F1, ROW_F2, NO], f32, kind="ExternalInput"
    ).ap()
    idx_d = nc.dram_tensor("idx", [P, S], mybir.dt.int32, kind="ExternalInput").ap()
    # aux layout per partition (f32): T[S,4] | onehot[S,6]
    aux_d = nc.dram_tensor("aux", [P, S * 10], f32, kind="ExternalInput").ap()
    # per-body [4,1] outputs stacked on rows
    out_d = nc.dram_tensor("out", [repeat * 4, 1], f32, kind="ExternalOutput").ap()

    with tile.TileContext(nc) as tc:
        with (
            tc.tile_pool(name="cpool", bufs=1) as cpool,
            tc.tile_pool(name="pool", bufs=2) as pool,
            tc.tile_pool(name="psum", bufs=2, space="PSUM") as psp,
        ):
            idx_t = cpool.tile([P, S], mybir.dt.int32)
            aux_t = cpool.tile([P, S * 10], f32)
            ones = cpool.tile([P, 1], f32)
            nc.sync.dma_start(out=idx_t[:], in_=idx_d[:])
            nc.sync.dma_start(out=aux_t[:], in_=aux_d[:])
            nc.vector.memset(ones[:], 1.0)
            # preload the act-table set holding BOTH Exp and Ln, so the
            # auto-insertion pass (greedy: exp->set0, ln->set5) doesn't
            # reload a table per activation (~1.3us each)
            from concourse.hw_specs import get_activation_tables

            A = mybir.ActivationFunctionType
            combined = next(
                i
                for i, (_nm, s) in enumerate(
                    get_activation_tables(nc.m.arch).items()
                )
                if A.Exp in s and A.Ln in s
            )
            nc.scalar.add_instruction(
                mybir.InstLoadActFuncSet(
                    name=nc.get_next_instruction_name(),
                    ins=[],
                    outs=[],
                    act_func_set_id=combined,
                )
            )
            for _rep in range(repeat):
                _emit_body(
                    nc,
                    pool,
                    psp,
                    predc,
                    idx_t,
                    aux_t,
                    ones,
                    out_d[_rep * 4 : (_rep + 1) * 4, :],
                    S,
                )

    nc.compile()
    return nc


def _softplus_np(x):
    x = np.asarray(x, dtype=np.float64)
    return np.log1p(np.exp(-np.abs(x))) + np.maximum(x, 0.0)


def _prepare_in_maps(pred_full, targets):
    """Shard inputs and build per-core index/aux tensors.

    Returns (S, in_maps, corrections) where corrections[i] is the
    deterministic pad contribution (box0, obj0, clsSP0) * npad for core i."""
    n = targets.shape[0]
    b = targets[:, 0].astype(np.int32)
    c = targets[:, 1].astype(np.int32)
    txywh = targets[:, 2:6].astype(np.float32)

    # per-layer global row index within a core's concatenated [TOT_ROWS, 11]
    rows_by_layer = []
    for l, (ny, nx) in enumerate(LAYERS):
        gx = np.clip(
            np.floor(np.float32(nx) * txywh[:, 0]).astype(np.int32), 0, nx - 1
        )
        gy = np.clip(
            np.floor(np.float32(ny) * txywh[:, 1]).astype(np.int32), 0, ny - 1
        )
        rows_by_layer.append(
            LAYER_BASE[l] + ((b % BPC) * NA * ny + gy) * nx + gx
        )

    core_of = b // BPC
    counts = np.bincount(core_of, minlength=NCORES)
    # slots hold (target, layer) pairs; one global ceil over 3*m
    S = max(1, -(-int(counts.max()) * 3 // P))
    C = S * P

    onehot = np.zeros((n, NCLS), dtype=np.float32)
    onehot[np.arange(n), np.clip(c, 0, NCLS - 1)] = 1.0

    in_maps = []
    corrections = []
    for i in range(NCORES):
        sel = np.nonzero(core_of == i)[0]
        m = len(sel)
        npad = C - 3 * m

        def pad_ps(a):
            """per-slot array [3m, ...] -> pad to [C, ...] -> [P, S, ...]"""
            out = np.zeros((C,) + a.shape[1:], dtype=a.dtype)
            out[: a.shape[0]] = a
            return np.ascontiguousarray(
                out.reshape((S, P) + a.shape[1:]).swapaxes(0, 1)
            )

        idx_flat = np.concatenate([r[sel] for r in rows_by_layer])  # [3m]
        T_flat = np.concatenate([txywh[sel]] * 3, axis=0)  # [3m, 4]
        W2_flat = np.concatenate([onehot[sel]] * 3, axis=0)  # [3m, 6]

        idx = pad_ps(idx_flat.reshape(-1, 1))[:, :, 0].astype(np.int32)  # [P, S]
        T_s = pad_ps(T_flat).reshape(P, -1)
        W2_s = pad_ps(W2_flat).reshape(P, -1)
        aux = np.concatenate([T_s, W2_s], axis=1).astype(np.float32)
        assert aux.shape == (P, 10 * S)

        shards = [
            np.ascontiguousarray(pred_full[l][i * BPC : (i + 1) * BPC]).reshape(
                -1, NO
            )
            for l in range(3)
        ]
        predc = np.concatenate(shards, axis=0)
        row0 = predc[0].astype(np.float64)  # pad slots gather this row
        corrections.append(
            (
                npad * np.abs(row0[0:4]).sum(),
                npad * _softplus_np(-row0[4]),
                npad * _softplus_np(row0[5:11]).sum(),
            )
        )
        predc = predc.reshape(ROW_F1, ROW_F2, NO)

        in_maps.append(
            {"idx": idx, "aux": np.ascontiguousarray(aux), "predc": predc}
        )

    return S, in_maps, corrections


def _combine(results, corrections, n):
    """Host reduction: 8 per-core [4,1] partials -> (loss, lbox, lobj, lcls)."""
    inv_n = 1.0 / max(1, n)
    box = obj = clssp = clsw2 = 0.0
    for r, (box0, obj0, cls0) in zip(results, corrections):
        q = r["out"][0:4, 0].astype(np.float64)
        box += q[0] - box0
        obj += q[1] - obj0
        clssp += q[2] - cls0
        clsw2 += q[3]
    lbox = np.float32(box * inv_n * BOX_GAIN)
    lobj = np.float32(obj * inv_n * DFL_GAIN)
    lcls = np.float32((clssp - clsw2) * inv_n * CLS_GAIN)
    loss = np.float32(lbox + lobj + lcls)
    return np.asarray([loss, lbox, lobj, lcls], dtype=np.float32)


def _run(pred_full, targets, trace=False, **run_kwargs):
    from concourse import bass_utils

    S, in_maps, corrections = _prepare_in_maps(pred_full, targets)
    if S not in _BUILD_CACHE:
        _BUILD_CACHE[S] = _build(S)
    nc = _BUILD_CACHE[S]
    res = bass_utils.run_bass_kernel_spmd(
        nc, in_maps, core_ids=list(range(NCORES)), trace=trace, **run_kwargs
    )
    out = _combine(res.results, corrections, targets.shape[0])
    return out, res


def kernel(**inputs) -> np.ndarray:
    pred_full = [
        np.asarray(inputs[f"pred{l}"], dtype=np.float32) for l in range(3)
    ]
    targets = np.asarray(inputs["targets"], dtype=np.float32)
    out, _ = _run(pred_full, targets, trace=False)
    return out
